# revision 16
# baseline (speedup 1.0000x reference)
"""Trainium2 Bass kernel for nn_AttentionLayer (B=64, S=2048, H=1024).

Computation (per batch b):
    c[b]      = hidden[b] @ W0_hid + b0                      # (H,)  tiny
    z[b,s]    = enc[b,s] @ W0_enc + c[b]                     # main matmul
    score[b,s]= w1 . tanh(z[b,s])        (+ b1, dropped: softmax shift-inv)
    attn      = softmax(where(mask, score, -inf))
    out[b]    = sum_s attn[b,s] * enc[b,s]

Sharding: pure data parallel, 8 batches per core on 8 cores; params
replicated. Masked rows are skipped entirely: the host packs each
batch's unmasked rows densely (pads get score bias -1e30 -> weight 0)
and pre-builds both layouts the device needs (natural [s,h] and
transposed [h,s]) so the device streams plain HWDGE loads - no
indirect gathers, no on-device transposes.

Batches are dealt to cores by descending chunk count (slot template):
slot k on every core has the same compile-time chunk count
slot_sizes[k] = the k-th rank-group maximum, so the single SPMD
program fits all cores with minimal padding. The host permutes
batches into slots and inverse-permutes the output rows.

Layout ("s-on-partitions"): per packed 128-row chunk,
  - z = encT_kc^T @ W0e accumulated over kc -> PSUM [128s, 1024h]
    (encT chunk is the stationary operand; W0e streams at N=512)
  - DVE adds c[slot] (preamble-computed, PE-broadcast to all rows)
  - ACT tanh -> th [128s, 1024h] bf16
  - score[128,1] = sum_h th*w1 via DVE mult + reduce (w1 pre-broadcast)
    Scores are born with s on partitions - no transposes, no score
    matmuls on the PE.
  - ACT exp with per-partition mask bias (no max subtraction needed:
    |score| <= ||w1||_1 ~ 26, exp fits fp32 with huge margin; softmax
    is shift-invariant so this is exact)
  - contribution: pc[1,1024] += p^T @ enc_nat and l[1,1] += p^T @ ones,
    M=1 matmuls accumulated in PSUM across all chunks of the batch.
  - batch end: out = pc / l.
PE work per chunk = 16 main MMs (N=512) + 3 tiny contrib MMs; the
rest rides on DVE/ACT/DMA in parallel. Contribution for chunk g is
emitted after the main matmuls of chunk g+2 (software pipeline).
"""

import os
import sys

import numpy as np

for _p in ("/opt/trn_rl_repo", "/root/.axon_site/_ro/trn_rl_repo"):
    if os.path.isdir(_p) and _p not in sys.path:
        sys.path.insert(0, _p)

B, S, H = 64, 2048, 1024
N_CORES = 8
BL = B // N_CORES  # 8 batch slots per core
NKC = H // 128     # 8 contraction chunks

LAG = 2            # contrib software-pipeline lag (chunks)
GW = 3             # chunks per batched load group

_CACHE = {}


def _build(slot_sizes):
    import concourse.bass as bass
    import concourse.bacc as bacc
    import concourse.tile as tile
    from concourse import mybir
    from contextlib import ExitStack

    F32 = mybir.dt.float32
    BF16 = mybir.dt.bfloat16
    AF = mybir.ActivationFunctionType
    ALU = mybir.AluOpType

    TOT = sum(slot_sizes)              # chunks per core
    off = [0]
    for n in slot_sizes:
        off.append(off[-1] + n)        # chunk offset of each slot

    nc = bacc.Bacc(trn_type="TRN2")

    encg_d = nc.dram_tensor("encg", [TOT * 128, H], BF16, kind="ExternalInput")
    encT_d = nc.dram_tensor("encTg", [TOT, 128, H], BF16, kind="ExternalInput")
    hidT_d = nc.dram_tensor("hidT", [128, NKC * BL], BF16, kind="ExternalInput")
    w0e_d = nc.dram_tensor("W0e", [H, H], BF16, kind="ExternalInput")
    w0h_d = nc.dram_tensor("W0h", [H, H], BF16, kind="ExternalInput")
    b0_d = nc.dram_tensor("b0", [1, H], BF16, kind="ExternalInput")
    w1_d = nc.dram_tensor("w1", [1, H], BF16, kind="ExternalInput")
    oneb_d = nc.dram_tensor("oneb", [1, 128], BF16, kind="ExternalInput")
    onec_d = nc.dram_tensor("onec", [128, 1], BF16, kind="ExternalInput")
    ebsel_d = nc.dram_tensor("ebsel", [BL, BL * 128], BF16, kind="ExternalInput")
    mb_d = nc.dram_tensor("mbias", [128, TOT], F32, kind="ExternalInput")
    out_d = nc.dram_tensor("out", [BL, H], F32, kind="ExternalOutput")

    with tile.TileContext(nc) as tc:
        with ExitStack() as ctx:
            persist = ctx.enter_context(tc.tile_pool(name="persist", bufs=1))
            zp = ctx.enter_context(
                tc.tile_pool(name="zp", bufs=2, space=bass.MemorySpace.PSUM))
            pcp = ctx.enter_context(
                tc.tile_pool(name="pcp", bufs=1, space=bass.MemorySpace.PSUM))
            lp = ctx.enter_context(
                tc.tile_pool(name="lp", bufs=1, space=bass.MemorySpace.PSUM))

            # persistent SBUF tensors. Emission order = queue order:
            # w0h first on sync (heads the preamble dependency chain),
            # then w0e, then the enc load stream.
            w0h_t = persist.tile([128, NKC, H], BF16, tag="w0h")
            nc.sync.dma_start(
                w0h_t[:], w0h_d[:].rearrange("(kc p) m -> p kc m", p=128))
            w0e = persist.tile([128, NKC, H], BF16, tag="w0e")
            nc.sync.dma_start(
                w0e[:], w0e_d[:].rearrange("(kc p) m -> p kc m", p=128))
            hidT = persist.tile([128, NKC * BL], BF16, tag="hidT")
            nc.gpsimd.dma_start(hidT[:], hidT_d[:])
            b0r = persist.tile([1, H], BF16, tag="b0r")
            nc.gpsimd.dma_start(b0r[:], b0_d[:])
            w1r = persist.tile([1, H], BF16, tag="w1r")
            nc.gpsimd.dma_start(w1r[:], w1_d[:])
            oneb = persist.tile([1, 128], BF16, tag="oneb")
            nc.gpsimd.dma_start(oneb[:], oneb_d[:])
            onec = persist.tile([128, 1], BF16, tag="onec")
            nc.gpsimd.dma_start(onec[:], onec_d[:])
            ebsel = persist.tile([BL, BL * 128], BF16, tag="ebsel")
            nc.gpsimd.dma_start(ebsel[:], ebsel_d[:])
            mbs = persist.tile([128, TOT], F32, tag="mbs")
            nc.gpsimd.dma_start(mbs[:], mb_d[:])
            cs_sb = persist.tile([BL, H], BF16, tag="cs_sb")
            w1R = persist.tile([128, H], BF16, tag="w1R")
            cbR = persist.tile([128, BL, H], F32, tag="cbR")

            # ---- preamble: cs[b,:] = hid[b] @ W0h + b0 (bf16), then
            # PE-broadcast to cbR[:, b, :] and w1 to w1R ----
            for nh in range(2):
                csp = zp.tile([128, 512], F32, tag="za" if nh == 0 else "zb")
                for kc in range(NKC):
                    nc.tensor.matmul(
                        csp[0:BL, :],
                        hidT[:, kc * BL:(kc + 1) * BL],
                        w0h_t[:, kc, nh * 512:(nh + 1) * 512],
                        start=(kc == 0), stop=False)
                nc.tensor.matmul(
                    csp[0:BL, :], oneb[0:1, 0:BL],
                    b0r[0:1, nh * 512:(nh + 1) * 512],
                    start=False, stop=True)
                nc.vector.tensor_copy(
                    cs_sb[:, nh * 512:(nh + 1) * 512], csp[0:BL, :])
            for nh in range(2):
                wp = zp.tile([128, 512], F32, tag="za" if nh == 0 else "zb")
                nc.tensor.matmul(
                    wp[:], oneb[0:1, :],
                    w1r[0:1, nh * 512:(nh + 1) * 512],
                    start=True, stop=True)
                nc.vector.tensor_copy(w1R[:, nh * 512:(nh + 1) * 512], wp[:])
            for b in range(BL):
                for nh in range(2):
                    cp = zp.tile([128, 512], F32, tag="za" if nh == 0 else "zb")
                    nc.tensor.matmul(
                        cp[:], ebsel[0:BL, b * 128:(b + 1) * 128],
                        cs_sb[0:BL, nh * 512:(nh + 1) * 512],
                        start=True, stop=True)
                    nc.vector.tensor_copy(
                        cbR[:, b, nh * 512:(nh + 1) * 512], cp[:])

            # ---- main pools ----
            encp = ctx.enter_context(tc.tile_pool(name="encp", bufs=4))
            encTp = ctx.enter_context(tc.tile_pool(name="encT", bufs=4))
            thp = ctx.enter_context(tc.tile_pool(name="th", bufs=2))
            zcp = ctx.enter_context(tc.tile_pool(name="zc", bufs=2))
            prodp = ctx.enter_context(tc.tile_pool(name="prod", bufs=2))
            scp = ctx.enter_context(tc.tile_pool(name="sc", bufs=2))
            pp = ctx.enter_context(tc.tile_pool(name="pp", bufs=4))
            outp = ctx.enter_context(tc.tile_pool(name="outp", bufs=2))
            lip = ctx.enter_context(tc.tile_pool(name="lip", bufs=2))

            bstate = {}
            # chunk list: (slot, j); global chunk index t = off[slot] + j
            chunks = [(b, j) for b in range(BL) for j in range(slot_sizes[b])]
            # load groups of up to GW chunks (batched HWDGE loads)
            groups = []
            grp_of = {}
            for b in range(BL):
                for j0 in range(0, slot_sizes[b], GW):
                    gsz = min(GW, slot_sizes[b] - j0)
                    for j in range(j0, j0 + gsz):
                        grp_of[(b, j)] = len(groups)
                    groups.append((b, j0, gsz))

            loaded = {}

            def emit_load(gi):
                b, j0, gsz = groups[gi]
                t0 = off[b] + j0
                nat = encp.tile([128, GW, H], BF16, tag="enc")
                nc.sync.dma_start(
                    nat[:, 0:gsz, :],
                    encg_d[t0 * 128:(t0 + gsz) * 128, :]
                    .rearrange("(c p) h -> p c h", p=128))
                tr = encTp.tile([128, GW, NKC, 128], BF16, tag="encT")
                nc.sync.dma_start(
                    tr[:, 0:gsz],
                    encT_d[t0:t0 + gsz, :, :].rearrange("c p h -> p c h"))
                loaded[gi] = (nat, tr, j0)

            def stage_front(b, j):
                """Main matmul, c-add, tanh, score, exp."""
                col = off[b] + j
                nat, tr, j0 = loaded[grp_of[(b, j)]]
                enc_nat = nat[:, j - j0, :]
                encT = tr[:, j - j0]
                za = zp.tile([128, 512], F32, tag="za")
                zb = zp.tile([128, 512], F32, tag="zb")
                zs = (za, zb)
                for kc in range(NKC):
                    for nh in range(2):
                        nc.tensor.matmul(
                            zs[nh][:],
                            encT[:, kc, :],
                            w0e[:, kc, nh * 512:(nh + 1) * 512],
                            start=(kc == 0), stop=(kc == NKC - 1))
                th = thp.tile([128, H], BF16, tag="th")
                zc = zcp.tile([128, H], F32, tag="zc")
                for nh in range(2):
                    nc.vector.tensor_add(
                        zc[:, nh * 512:(nh + 1) * 512], zs[nh][:],
                        cbR[:, b, nh * 512:(nh + 1) * 512])
                    nc.scalar.activation(
                        th[:, nh * 512:(nh + 1) * 512],
                        zc[:, nh * 512:(nh + 1) * 512], AF.Tanh)
                prod = prodp.tile([128, H], BF16, tag="prod")
                score = scp.tile([128, 1], F32, tag="score")
                nc.gpsimd.tensor_tensor(
                    out=prod[:], in0=th[:], in1=w1R[:], op=ALU.mult)
                nc.vector.tensor_reduce(
                    out=score[:], in_=prod[:],
                    axis=mybir.AxisListType.X, op=ALU.add)
                p = pp.tile([128, 1], BF16, tag="p")
                nc.scalar.activation(p[:], score[:], AF.Exp,
                                     bias=mbs[:, col:col + 1])
                return enc_nat, p

            def stage_contrib(b, j, enc_nat, p):
                """Accumulate pc += p^T @ enc_nat, l += p^T @ ones."""
                if j == 0:
                    pc = pcp.tile([1, H], F32, tag="pc")
                    ls = lp.tile([1, 1], F32, tag="ls")
                    bstate[b] = (pc, ls)
                pc, ls = bstate[b]
                first, last = (j == 0), (j == slot_sizes[b] - 1)
                for nh in range(2):
                    nc.tensor.matmul(
                        pc[:, nh * 512:(nh + 1) * 512],
                        p[:], enc_nat[:, nh * 512:(nh + 1) * 512],
                        start=first, stop=last)
                nc.tensor.matmul(ls[:], p[:], onec[:], start=first, stop=last)
                if last:
                    linv = lip.tile([1, 1], F32, tag="linv")
                    nc.vector.reciprocal(linv[:], ls[:])
                    outt = outp.tile([1, H], F32, tag="outt")
                    nc.vector.tensor_tensor(
                        out=outt[:], in0=pc[:],
                        in1=linv[:].to_broadcast([1, H]), op=ALU.mult)
                    nc.gpsimd.dma_start(out_d[b:b + 1, :], outt[:])
                    del bstate[b]

            PRE = 2  # load-group prefetch depth
            pending = []
            last_gi = -1
            for g, (b, j) in enumerate(chunks):
                gi = grp_of[(b, j)]
                if gi != last_gi:
                    if gi == 0:
                        for k in range(min(PRE + 1, len(groups))):
                            emit_load(k)
                    else:
                        if gi + PRE < len(groups):
                            emit_load(gi + PRE)
                        loaded.pop(gi - 1, None)
                    last_gi = gi
                enc_nat, p = stage_front(b, j)
                pending.append((b, j, enc_nat, p))
                if g >= LAG:
                    stage_contrib(*pending.pop(0))
            while pending:
                stage_contrib(*pending.pop(0))

    nc.compile()
    return nc


def _get_nc(slot_sizes):
    key = tuple(slot_sizes)
    if key not in _CACHE:
        _CACHE[key] = _build(key)
    return _CACHE[key]


def _prep(hidden, enc_seq, mask, W0, b0, w1):
    import ml_dtypes
    bf = ml_dtypes.bfloat16

    mask = np.asarray(mask).astype(bool)
    enc = np.asarray(enc_seq)
    hid = np.asarray(hidden).reshape(B, H).astype(np.float32)
    W0 = np.asarray(W0, dtype=np.float32)
    w0e = np.ascontiguousarray(W0[:H].astype(bf))
    w0h = np.ascontiguousarray(W0[H:].astype(bf))
    b0r = np.asarray(b0, dtype=np.float32).reshape(1, H).astype(bf)
    w1r = np.asarray(w1).reshape(1, H).astype(np.float32).astype(bf)
    oneb = np.ones((1, 128), dtype=np.float32).astype(bf)
    onec = np.ones((128, 1), dtype=np.float32).astype(bf)
    ebsel = np.zeros((BL, BL * 128), dtype=np.float32)
    for b in range(BL):
        ebsel[b, b * 128:(b + 1) * 128] = 1.0
    ebsel = ebsel.astype(bf)

    counts = mask.sum(axis=1)
    nch_b = np.maximum(1, -(-counts // 128)).astype(int)
    # deal batches to cores by descending chunk count: rank r -> core r%8,
    # slot r//8. slot k size = max chunk count in rank group k.
    order = np.argsort(-nch_b, kind="stable")
    slot_sizes = tuple(
        int(nch_b[order[k * N_CORES:(k + 1) * N_CORES]].max())
        for k in range(BL))
    TOT = sum(slot_sizes)
    off = [0]
    for n in slot_sizes:
        off.append(off[-1] + n)

    batch_of = [[0] * BL for _ in range(N_CORES)]  # [core][slot] -> batch
    for r, gb in enumerate(order):
        batch_of[r % N_CORES][r // N_CORES] = int(gb)

    maps = []
    for c in range(N_CORES):
        mybatches = batch_of[c]
        hidc = hid[mybatches]  # [BL, H] in slot order
        hidT = np.ascontiguousarray(
            hidc.reshape(BL, NKC, 128).transpose(2, 1, 0)
            .reshape(128, NKC * BL).astype(bf))
        mbc = np.full((128, TOT), -1e30, dtype=np.float32)
        encg = np.zeros((TOT * 128, H), dtype=bf)
        for s, gb in enumerate(mybatches):
            rows = np.flatnonzero(mask[gb])
            cnt = len(rows)
            sp_s = slot_sizes[s] * 128
            encg[off[s] * 128:off[s] * 128 + cnt] = enc[gb, rows].astype(bf)
            mbf = np.full(sp_s, -1e30, dtype=np.float32)
            mbf[:cnt] = 0.0
            mbc[:, off[s]:off[s + 1]] = mbf.reshape(slot_sizes[s], 128).T
        # encTg[t, p, kc*128+s] = encg[t*128+s, kc*128+p]
        encTg = np.ascontiguousarray(
            encg.reshape(TOT, 128, NKC, 128)
            .transpose(0, 3, 2, 1).reshape(TOT, 128, H))
        m = {"encg": encg, "encTg": encTg, "hidT": hidT,
             "W0e": w0e, "W0h": w0h, "b0": b0r, "w1": w1r,
             "oneb": oneb, "onec": onec, "ebsel": ebsel,
             "mbias": mbc}
        maps.append(m)
    return maps, slot_sizes, batch_of


def _run(in_maps, slot_sizes, batch_of, **kwargs):
    from concourse.bass_utils import run_bass_kernel_spmd
    nc = _get_nc(slot_sizes)
    res = run_bass_kernel_spmd(nc, in_maps, list(range(N_CORES)), **kwargs)
    out = np.empty((B, H), dtype=np.float32)
    for c in range(N_CORES):
        oc = res.results[c]["out"]
        for s in range(BL):
            out[batch_of[c][s]] = oc[s]
    return out, res


def kernel(hidden, enc_seq, mask, W0, b0, w1, b1):
    # b1 shifts every score equally -> cancelled by softmax; unused.
    in_maps, slot_sizes, batch_of = _prep(
        hidden, enc_seq, mask, W0, b0, w1)
    out, _ = _run(in_maps, slot_sizes, batch_of)
    return out


def kernel_profiled(hidden, enc_seq, mask, W0, b0, w1, b1, **kwargs):
    in_maps, slot_sizes, batch_of = _prep(
        hidden, enc_seq, mask, W0, b0, w1)
    out, res = _run(in_maps, slot_sizes, batch_of, trace=True, **kwargs)
    return out, res


# revision 17
# speedup vs baseline: 1.1721x; 1.1721x over previous
"""Trainium2 Bass kernel for nn_AttentionLayer (B=64, S=2048, H=1024).

Computation (per batch b):
    c[b]      = hidden[b] @ W0_hid + b0                      # (H,)  tiny
    z[b,s]    = enc[b,s] @ W0_enc + c[b]                     # main matmul
    score[b,s]= w1 . tanh(z[b,s])        (+ b1, dropped: softmax shift-inv)
    attn      = softmax(where(mask, score, -inf))
    out[b]    = sum_s attn[b,s] * enc[b,s]

Sharding: pure data parallel, 8 batches per core on 8 cores; params
replicated. Masked rows are skipped entirely: the host packs each
batch's unmasked rows densely (pads get score bias -1e30 -> weight 0)
and pre-builds both layouts the device needs (natural [s,h] and
transposed [h,s]) so the device streams plain HWDGE loads - no
indirect gathers, no on-device transposes.

Batches are dealt to cores by descending chunk count (slot template):
slot k on every core has the same compile-time chunk count
slot_sizes[k] = the k-th rank-group maximum, so the single SPMD
program fits all cores with minimal padding. The host permutes
batches into slots and inverse-permutes the output rows.

Layout ("s-on-partitions"): per packed 128-row chunk,
  - z = encT_kc^T @ W0e accumulated over kc -> PSUM [128s, 1024h]
    (encT chunk is the stationary operand; W0e streams at N=512)
  - DVE adds c[slot] (preamble-computed, PE-broadcast to all rows)
  - ACT tanh -> th [128s, 1024h] bf16
  - score[128,1] = sum_h th*w1 via DVE mult + reduce (w1 pre-broadcast)
    Scores are born with s on partitions - no transposes, no score
    matmuls on the PE.
  - ACT exp with per-partition mask bias (no max subtraction needed:
    |score| <= ||w1||_1 ~ 26, exp fits fp32 with huge margin; softmax
    is shift-invariant so this is exact)
  - contribution: pc[1,1024] += p^T @ enc_nat and l[1,1] += p^T @ ones,
    M=1 matmuls accumulated in PSUM across all chunks of the batch.
  - batch end: out = pc / l.
PE work per chunk = 16 main MMs (N=512) + 3 tiny contrib MMs; the
rest rides on DVE/ACT/DMA in parallel. Contribution for chunk g is
emitted after the main matmuls of chunk g+2 (software pipeline).
"""

import os
import sys

import numpy as np

for _p in ("/opt/trn_rl_repo", "/root/.axon_site/_ro/trn_rl_repo"):
    if os.path.isdir(_p) and _p not in sys.path:
        sys.path.insert(0, _p)

B, S, H = 64, 2048, 1024
N_CORES = 8
BL = B // N_CORES  # 8 batch slots per core
NKC = H // 128     # 8 contraction chunks

LAG = 2            # contrib software-pipeline lag (chunks)
GW = 3             # chunks per batched load group

_CACHE = {}


def _build(slot_sizes):
    import concourse.bass as bass
    import concourse.bacc as bacc
    import concourse.tile as tile
    from concourse import mybir
    from contextlib import ExitStack

    F32 = mybir.dt.float32
    BF16 = mybir.dt.bfloat16
    AF = mybir.ActivationFunctionType
    ALU = mybir.AluOpType

    TOT = sum(slot_sizes)              # chunks per core
    off = [0]
    for n in slot_sizes:
        off.append(off[-1] + n)        # chunk offset of each slot

    nc = bacc.Bacc(trn_type="TRN2")

    encg_d = nc.dram_tensor("encg", [TOT * 128, H], BF16, kind="ExternalInput")
    encT_d = nc.dram_tensor("encTg", [TOT, 128, H], BF16, kind="ExternalInput")
    hidT_d = nc.dram_tensor("hidT", [128, NKC * BL], BF16, kind="ExternalInput")
    w0e_d = nc.dram_tensor("W0e", [H, H], BF16, kind="ExternalInput")
    w0h_d = nc.dram_tensor("W0h", [H, H], BF16, kind="ExternalInput")
    b0_d = nc.dram_tensor("b0", [1, H], BF16, kind="ExternalInput")
    w1_d = nc.dram_tensor("w1", [1, H], BF16, kind="ExternalInput")
    oneb_d = nc.dram_tensor("oneb", [1, 128], BF16, kind="ExternalInput")
    onec_d = nc.dram_tensor("onec", [128, 1], BF16, kind="ExternalInput")
    ebsel_d = nc.dram_tensor("ebsel", [BL, BL * 128], BF16, kind="ExternalInput")
    mb_d = nc.dram_tensor("mbias", [128, TOT], F32, kind="ExternalInput")
    out_d = nc.dram_tensor("out", [BL, H], F32, kind="ExternalOutput")

    with tile.TileContext(nc) as tc:
        with ExitStack() as ctx:
            persist = ctx.enter_context(tc.tile_pool(name="persist", bufs=1))
            zp = ctx.enter_context(
                tc.tile_pool(name="zp", bufs=2, space=bass.MemorySpace.PSUM))
            pcp = ctx.enter_context(
                tc.tile_pool(name="pcp", bufs=1, space=bass.MemorySpace.PSUM))
            lp = ctx.enter_context(
                tc.tile_pool(name="lp", bufs=1, space=bass.MemorySpace.PSUM))

            # persistent SBUF tensors. Emission order = queue order:
            # w0h first on sync (heads the preamble dependency chain),
            # then w0e, then the enc load stream.
            w0h_t = persist.tile([128, NKC, H], BF16, tag="w0h")
            nc.sync.dma_start(
                w0h_t[:], w0h_d[:].rearrange("(kc p) m -> p kc m", p=128))
            w0e = persist.tile([128, NKC, H], BF16, tag="w0e")
            nc.sync.dma_start(
                w0e[:], w0e_d[:].rearrange("(kc p) m -> p kc m", p=128))
            hidT = persist.tile([128, NKC * BL], BF16, tag="hidT")
            nc.gpsimd.dma_start(hidT[:], hidT_d[:])
            b0r = persist.tile([1, H], BF16, tag="b0r")
            nc.gpsimd.dma_start(b0r[:], b0_d[:])
            w1r = persist.tile([1, H], BF16, tag="w1r")
            nc.gpsimd.dma_start(w1r[:], w1_d[:])
            oneb = persist.tile([1, 128], BF16, tag="oneb")
            nc.gpsimd.dma_start(oneb[:], oneb_d[:])
            onec = persist.tile([128, 1], BF16, tag="onec")
            nc.gpsimd.dma_start(onec[:], onec_d[:])
            ebsel = persist.tile([BL, BL * 128], BF16, tag="ebsel")
            nc.gpsimd.dma_start(ebsel[:], ebsel_d[:])
            mbs = persist.tile([128, TOT], F32, tag="mbs")
            nc.gpsimd.dma_start(mbs[:], mb_d[:])
            cs_sb = persist.tile([BL, H], BF16, tag="cs_sb")
            w1R = persist.tile([128, H], BF16, tag="w1R")
            cbR = persist.tile([128, BL, H], F32, tag="cbR")

            # ---- preamble: cs[b,:] = hid[b] @ W0h + b0 (bf16), then
            # PE-broadcast to cbR[:, b, :] and w1 to w1R ----
            for nh in range(2):
                csp = zp.tile([128, 512], F32, tag="za" if nh == 0 else "zb")
                for kc in range(NKC):
                    nc.tensor.matmul(
                        csp[0:BL, :],
                        hidT[:, kc * BL:(kc + 1) * BL],
                        w0h_t[:, kc, nh * 512:(nh + 1) * 512],
                        start=(kc == 0), stop=False)
                nc.tensor.matmul(
                    csp[0:BL, :], oneb[0:1, 0:BL],
                    b0r[0:1, nh * 512:(nh + 1) * 512],
                    start=False, stop=True)
                nc.vector.tensor_copy(
                    cs_sb[:, nh * 512:(nh + 1) * 512], csp[0:BL, :])
            for nh in range(2):
                wp = zp.tile([128, 512], F32, tag="za" if nh == 0 else "zb")
                nc.tensor.matmul(
                    wp[:], oneb[0:1, :],
                    w1r[0:1, nh * 512:(nh + 1) * 512],
                    start=True, stop=True)
                nc.vector.tensor_copy(w1R[:, nh * 512:(nh + 1) * 512], wp[:])
            for b in range(BL):
                for nh in range(2):
                    cp = zp.tile([128, 512], F32, tag="za" if nh == 0 else "zb")
                    nc.tensor.matmul(
                        cp[:], ebsel[0:BL, b * 128:(b + 1) * 128],
                        cs_sb[0:BL, nh * 512:(nh + 1) * 512],
                        start=True, stop=True)
                    nc.vector.tensor_copy(
                        cbR[:, b, nh * 512:(nh + 1) * 512], cp[:])

            # ---- main pools ----
            encp = ctx.enter_context(tc.tile_pool(name="encp", bufs=4))
            encTp = ctx.enter_context(tc.tile_pool(name="encT", bufs=4))
            thp = ctx.enter_context(tc.tile_pool(name="th", bufs=2))
            zcp = ctx.enter_context(tc.tile_pool(name="zc", bufs=2))
            prodp = ctx.enter_context(tc.tile_pool(name="prod", bufs=2))
            scp = ctx.enter_context(tc.tile_pool(name="sc", bufs=2))
            pp = ctx.enter_context(tc.tile_pool(name="pp", bufs=4))
            outp = ctx.enter_context(tc.tile_pool(name="outp", bufs=2))
            lip = ctx.enter_context(tc.tile_pool(name="lip", bufs=2))

            bstate = {}
            # chunk list: (slot, j); global chunk index t = off[slot] + j
            chunks = [(b, j) for b in range(BL) for j in range(slot_sizes[b])]
            # load groups of up to GW chunks (batched HWDGE loads)
            groups = []
            grp_of = {}
            for b in range(BL):
                for j0 in range(0, slot_sizes[b], GW):
                    gsz = min(GW, slot_sizes[b] - j0)
                    for j in range(j0, j0 + gsz):
                        grp_of[(b, j)] = len(groups)
                    groups.append((b, j0, gsz))

            loaded = {}

            def emit_load(gi):
                b, j0, gsz = groups[gi]
                t0 = off[b] + j0
                nat = encp.tile([128, GW, H], BF16, tag="enc")
                nc.sync.dma_start(
                    nat[:, 0:gsz, :],
                    encg_d[t0 * 128:(t0 + gsz) * 128, :]
                    .rearrange("(c p) h -> p c h", p=128))
                tr = encTp.tile([128, GW, NKC, 128], BF16, tag="encT")
                nc.sync.dma_start(
                    tr[:, 0:gsz],
                    encT_d[t0:t0 + gsz, :, :].rearrange("c p h -> p c h"))
                loaded[gi] = (nat, tr, j0)

            def stage_front(b, j):
                """Main matmul, c-add, tanh, score, exp."""
                col = off[b] + j
                nat, tr, j0 = loaded[grp_of[(b, j)]]
                enc_nat = nat[:, j - j0, :]
                encT = tr[:, j - j0]
                za = zp.tile([128, 512], F32, tag="za")
                zb = zp.tile([128, 512], F32, tag="zb")
                zs = (za, zb)
                for kc in range(NKC):
                    for nh in range(2):
                        nc.tensor.matmul(
                            zs[nh][:],
                            encT[:, kc, :],
                            w0e[:, kc, nh * 512:(nh + 1) * 512],
                            start=(kc == 0), stop=(kc == NKC - 1))
                th = thp.tile([128, H], BF16, tag="th")
                zc = zcp.tile([128, H], F32, tag="zc")
                for nh in range(2):
                    nc.vector.tensor_add(
                        zc[:, nh * 512:(nh + 1) * 512], zs[nh][:],
                        cbR[:, b, nh * 512:(nh + 1) * 512])
                    nc.scalar.activation(
                        th[:, nh * 512:(nh + 1) * 512],
                        zc[:, nh * 512:(nh + 1) * 512], AF.Tanh)
                prod = prodp.tile([128, H], BF16, tag="prod")
                score = scp.tile([128, 1], F32, tag="score")
                nc.vector.tensor_tensor(
                    out=prod[:], in0=th[:], in1=w1R[:], op=ALU.mult)
                nc.vector.tensor_reduce(
                    out=score[:], in_=prod[:],
                    axis=mybir.AxisListType.X, op=ALU.add)
                p = pp.tile([128, 1], BF16, tag="p")
                nc.scalar.activation(p[:], score[:], AF.Exp,
                                     bias=mbs[:, col:col + 1])
                return enc_nat, p

            def stage_contrib(b, j, enc_nat, p):
                """Accumulate pc += p^T @ enc_nat, l += p^T @ ones."""
                if j == 0:
                    pc = pcp.tile([1, H], F32, tag="pc")
                    ls = lp.tile([1, 1], F32, tag="ls")
                    bstate[b] = (pc, ls)
                pc, ls = bstate[b]
                first, last = (j == 0), (j == slot_sizes[b] - 1)
                for nh in range(2):
                    nc.tensor.matmul(
                        pc[:, nh * 512:(nh + 1) * 512],
                        p[:], enc_nat[:, nh * 512:(nh + 1) * 512],
                        start=first, stop=last)
                nc.tensor.matmul(ls[:], p[:], onec[:], start=first, stop=last)
                if last:
                    linv = lip.tile([1, 1], F32, tag="linv")
                    nc.vector.reciprocal(linv[:], ls[:])
                    outt = outp.tile([1, H], F32, tag="outt")
                    nc.vector.tensor_tensor(
                        out=outt[:], in0=pc[:],
                        in1=linv[:].to_broadcast([1, H]), op=ALU.mult)
                    nc.gpsimd.dma_start(out_d[b:b + 1, :], outt[:])
                    del bstate[b]

            PRE = 2  # load-group prefetch depth
            pending = []
            last_gi = -1
            for g, (b, j) in enumerate(chunks):
                gi = grp_of[(b, j)]
                if gi != last_gi:
                    if gi == 0:
                        for k in range(min(PRE + 1, len(groups))):
                            emit_load(k)
                    else:
                        if gi + PRE < len(groups):
                            emit_load(gi + PRE)
                        loaded.pop(gi - 1, None)
                    last_gi = gi
                enc_nat, p = stage_front(b, j)
                pending.append((b, j, enc_nat, p))
                if g >= LAG:
                    stage_contrib(*pending.pop(0))
            while pending:
                stage_contrib(*pending.pop(0))

    nc.compile()
    return nc


def _get_nc(slot_sizes):
    key = tuple(slot_sizes)
    if key not in _CACHE:
        _CACHE[key] = _build(key)
    return _CACHE[key]


def _prep(hidden, enc_seq, mask, W0, b0, w1):
    import ml_dtypes
    bf = ml_dtypes.bfloat16

    mask = np.asarray(mask).astype(bool)
    enc = np.asarray(enc_seq)
    hid = np.asarray(hidden).reshape(B, H).astype(np.float32)
    W0 = np.asarray(W0, dtype=np.float32)
    w0e = np.ascontiguousarray(W0[:H].astype(bf))
    w0h = np.ascontiguousarray(W0[H:].astype(bf))
    b0r = np.asarray(b0, dtype=np.float32).reshape(1, H).astype(bf)
    w1r = np.asarray(w1).reshape(1, H).astype(np.float32).astype(bf)
    oneb = np.ones((1, 128), dtype=np.float32).astype(bf)
    onec = np.ones((128, 1), dtype=np.float32).astype(bf)
    ebsel = np.zeros((BL, BL * 128), dtype=np.float32)
    for b in range(BL):
        ebsel[b, b * 128:(b + 1) * 128] = 1.0
    ebsel = ebsel.astype(bf)

    counts = mask.sum(axis=1)
    nch_b = np.maximum(1, -(-counts // 128)).astype(int)
    # deal batches to cores by descending chunk count: rank r -> core r%8,
    # slot r//8. slot k size = max chunk count in rank group k.
    order = np.argsort(-nch_b, kind="stable")
    slot_sizes = tuple(
        int(nch_b[order[k * N_CORES:(k + 1) * N_CORES]].max())
        for k in range(BL))
    TOT = sum(slot_sizes)
    off = [0]
    for n in slot_sizes:
        off.append(off[-1] + n)

    batch_of = [[0] * BL for _ in range(N_CORES)]  # [core][slot] -> batch
    for r, gb in enumerate(order):
        batch_of[r % N_CORES][r // N_CORES] = int(gb)

    maps = []
    for c in range(N_CORES):
        mybatches = batch_of[c]
        hidc = hid[mybatches]  # [BL, H] in slot order
        hidT = np.ascontiguousarray(
            hidc.reshape(BL, NKC, 128).transpose(2, 1, 0)
            .reshape(128, NKC * BL).astype(bf))
        mbc = np.full((128, TOT), -1e30, dtype=np.float32)
        encg = np.zeros((TOT * 128, H), dtype=bf)
        for s, gb in enumerate(mybatches):
            rows = np.flatnonzero(mask[gb])
            cnt = len(rows)
            sp_s = slot_sizes[s] * 128
            encg[off[s] * 128:off[s] * 128 + cnt] = enc[gb, rows].astype(bf)
            mbf = np.full(sp_s, -1e30, dtype=np.float32)
            mbf[:cnt] = 0.0
            mbc[:, off[s]:off[s + 1]] = mbf.reshape(slot_sizes[s], 128).T
        # encTg[t, p, kc*128+s] = encg[t*128+s, kc*128+p]
        encTg = np.ascontiguousarray(
            encg.reshape(TOT, 128, NKC, 128)
            .transpose(0, 3, 2, 1).reshape(TOT, 128, H))
        m = {"encg": encg, "encTg": encTg, "hidT": hidT,
             "W0e": w0e, "W0h": w0h, "b0": b0r, "w1": w1r,
             "oneb": oneb, "onec": onec, "ebsel": ebsel,
             "mbias": mbc}
        maps.append(m)
    return maps, slot_sizes, batch_of


def _run(in_maps, slot_sizes, batch_of, **kwargs):
    from concourse.bass_utils import run_bass_kernel_spmd
    nc = _get_nc(slot_sizes)
    res = run_bass_kernel_spmd(nc, in_maps, list(range(N_CORES)), **kwargs)
    out = np.empty((B, H), dtype=np.float32)
    for c in range(N_CORES):
        oc = res.results[c]["out"]
        for s in range(BL):
            out[batch_of[c][s]] = oc[s]
    return out, res


def kernel(hidden, enc_seq, mask, W0, b0, w1, b1):
    # b1 shifts every score equally -> cancelled by softmax; unused.
    in_maps, slot_sizes, batch_of = _prep(
        hidden, enc_seq, mask, W0, b0, w1)
    out, _ = _run(in_maps, slot_sizes, batch_of)
    return out


def kernel_profiled(hidden, enc_seq, mask, W0, b0, w1, b1, **kwargs):
    in_maps, slot_sizes, batch_of = _prep(
        hidden, enc_seq, mask, W0, b0, w1)
    out, res = _run(in_maps, slot_sizes, batch_of, trace=True, **kwargs)
    return out, res
